# revision 60
# baseline (speedup 1.0000x reference)
"""Trainium2 Bass kernel for a 3-layer GCN (DeepGRL) on 8 NeuronCores.

Strategy (per the dst-partitioned sharding plan):
  - Nodes are sharded contiguously across the 8 cores; edges are owned by the
    core that owns their destination node.
  - Per layer:  h = a @ W  (dense matmul on PE, per-core own nodes),
    u = dinv * h is written to a DRAM table and AllGather'ed so every core
    holds the full [N, F] table.
  - Aggregation out_i = dinv_i * (sum_{e: dst=i} u[src_e] + u_i) + b is done
    per 128-dst-node block: edge source rows are fetched with the SWDGE
    dma_gather instruction (128 rows per chunk land on 128 partitions), and a
    one-hot "segment matrix" S (built on the vector engine from the dst-local
    ids with an is_equal compare against an iota row) maps edges to dst rows
    via a PE matmul accumulating in PSUM.
  - BatchNorm batch statistics (sum / sum-of-squares per feature) are computed
    with ones-vector matmuls and AllReduce'd across cores; BN apply + ReLU is
    fused into one scalar-engine activation during the transpose back to the
    feature-major layout the next layer's matmul needs.

dma_gather indices are int16, so the gathered table is addressed in two
halves (rows < HALF and rows >= HALF); every dst block's edge list is split
into a "lo" and a "hi" sublist, each padded to a multiple of 128.

Execution path: the compiled module and the jax/PJRT executable are cached
per (shapes, edge-structure) key; input arrays are content-hashed and staged
on device once, so repeat calls only dispatch the NEFF and fetch the output.
"""

import hashlib
import math
from contextlib import ExitStack

import numpy as np

import concourse.bacc as bacc
import concourse.bass as bass
import concourse.mybir as mybir
import concourse.tile as tile
from concourse import library_config

P = 128
F32 = mybir.dt.float32
BF16 = mybir.dt.bfloat16
FP8 = mybir.dt.float8e4  # e4m3: required by the DoubleRow matmul perf mode
I16 = mybir.dt.int16
I8 = mybir.dt.int8
AF = mybir.ActivationFunctionType
ALU = mybir.AluOpType


# ----------------------------------------------------------------------------
# Host-side graph preprocessing
# ----------------------------------------------------------------------------
def make_plan(edge_index, N, n_cores=8, gb=2):
    """Partition edges by destination core, build per-core gather index /
    segment-id arrays (compile-time constants of the kernel).

    Gather layout: the u table is addressed in PAIRS of adjacent rows (one
    512-byte dma_gather element covers rows 2i and 2i+1), which halves the
    descriptor count of the random gathers (the dominant cost) and keeps the
    int16 index range (<= rows_total/2 = 25088).  Each edge therefore carries
    a parity bit; the one-hot segment matrix is built per (chunk, parity).
    Chunk counts are exact per dst block (max over the 8 cores, since the
    BIR is shared)."""
    src = np.asarray(edge_index[0], dtype=np.int64)
    dst = np.asarray(edge_index[1], dtype=np.int64)
    E = src.shape[0]

    indeg = np.bincount(dst, minlength=N).astype(np.float64)
    deg = indeg + 1.0  # self loop
    dinv = (1.0 / np.sqrt(deg)).astype(np.float32)

    npc = N // n_cores
    assert npc * n_cores == N
    nblk = math.ceil(npc / P)
    npc_pad = nblk * P

    # ---- load-balance: nodes are assigned to (core, block, slot) by an LPT
    # greedy on in-degree over ALL blocks of all cores (the host un-permutes
    # the output, so node->core ownership is free).  Every block ends up
    # with ~E/(n_cores*nblk) edges, so the shared per-block chunk counts
    # (maxed over cores) are tight and the per-core work is balanced.
    import heapq

    core_of = np.empty(N, np.int64)  # node -> owning core
    pos_of = np.empty(N, np.int64)   # node -> slot position in its core
    degs = indeg.astype(np.int64)
    order_d = np.argsort(-degs, kind="stable")
    heap = [(0, g, 0) for g in range(n_cores * nblk)]  # (load, gblock, used)
    heapq.heapify(heap)
    for i in order_d:
        while True:
            load, g, used = heapq.heappop(heap)
            if used < P:
                break
        core_of[i] = g // nblk
        pos_of[i] = (g % nblk) * P + used
        heapq.heappush(heap, (load + degs[i], g, used + 1))

    # source row in the padded, permuted global table layout
    src_row = core_of[src] * npc_pad + pos_of[src]
    pair = src_row // 2
    parity = src_row % 2

    dst_core = core_of[dst]
    dpos = pos_of[dst]
    gblk = dst_core * nblk + dpos // P  # global block id, 0..n_cores*nblk-1
    d_in_blk = dpos % P

    order = np.lexsort((pair, gblk))
    pair_s = pair[order]
    par_s = parity[order]
    dl_s = d_in_blk[order]

    counts = np.bincount(gblk, minlength=n_cores * nblk).reshape(
        n_cores, nblk)
    # chunks per dst block: shared across cores (same BIR), exact otherwise
    cnt_blk = np.maximum(1, -(-counts // P)).max(axis=0)  # [nblk]
    chunk_start = np.concatenate([[0], np.cumsum(cnt_blk)])  # [nblk+1]
    total_chunks = int(chunk_start[-1])

    # per-core slot arrays (slot = chunk position; sentinel dl=300, idx=0)
    idx_all = np.zeros((n_cores, total_chunks * P), np.int16)
    dle = np.full((n_cores, total_chunks * P), 300.0, np.float32)
    dlo = np.full((n_cores, total_chunks * P), 300.0, np.float32)
    edge_start = np.concatenate([[0], np.cumsum(counts.reshape(-1))])
    for r in range(n_cores):
        for b in range(nblk):
            g = r * nblk + b
            n_e = counts[r, b]
            s0 = edge_start[g]
            o0 = chunk_start[b] * P
            sl = slice(o0, o0 + n_e)
            idx_all[r, sl] = pair_s[s0:s0 + n_e]
            pe = par_s[s0:s0 + n_e]
            dd = dl_s[s0:s0 + n_e]
            dle[r, sl] = np.where(pe == 0, dd, 300.0)
            dlo[r, sl] = np.where(pe == 1, dd, 300.0)

    # gather groups of up to `gb` blocks: (b0, g, cstart, g_chunks)
    groups = []
    b0 = 0
    while b0 < nblk:
        g = min(gb, nblk - b0)
        cstart = int(chunk_start[b0])
        g_chunks = int(chunk_start[b0 + g] - chunk_start[b0])
        groups.append((b0, g, cstart, g_chunks))
        b0 += g

    # int16 wrap layout per gather: idx i -> [i % 16, i // 16], replicated
    # to 128 partitions (8 groups of 16); one wrapped strip per group
    idx_w = np.zeros((n_cores, P, total_chunks * 8), np.int16)
    for r in range(n_cores):
        for (_, _, cstart, g_chunks) in groups:
            ids = idx_all[r, cstart * P:(cstart + g_chunks) * P]
            w = ids.reshape(-1, 16).T  # [16, n/16]
            idx_w[r][:, cstart * 8:cstart * 8 + w.shape[1]] = np.tile(
                w, (8, 1))

    # dl array: per chunk two columns (even parity, odd parity)
    dl_arr = np.empty((n_cores, P, 2 * total_chunks), np.float32)
    dl_arr[:, :, 0::2] = dle.reshape(n_cores, total_chunks, P).transpose(
        0, 2, 1)
    dl_arr[:, :, 1::2] = dlo.reshape(n_cores, total_chunks, P).transpose(
        0, 2, 1)

    # int8 form for the DVE one-hot compare (sentinel -1)
    dl8 = np.where(dl_arr == 300.0, -1.0, dl_arr).astype(np.int8)

    dinv_own = np.zeros((n_cores, npc_pad), dtype=np.float32)
    dinv_own[core_of, pos_of] = dinv
    dinv_own = dinv_own.reshape(n_cores, nblk, P).transpose(0, 2, 1)

    return dict(
        n_cores=n_cores,
        N=N,
        E=E,
        npc=npc,
        nblk=nblk,
        npc_pad=npc_pad,
        cnt_blk=cnt_blk,
        chunk_start=chunk_start,
        total_chunks=total_chunks,
        groups=groups,
        idx=idx_w,
        dl=dl_arr,
        dinv_own=dinv_own,
        pos_of=pos_of,
        core_of=core_of,
        dl8=dl8,
    )


# ----------------------------------------------------------------------------
# Kernel builder (same BIR for all cores; per-core data via input tensors)
# ----------------------------------------------------------------------------
def build_kernel(plan, DIN, F1, F2, F3, skip=(), repeat=1, table8=True):
    n_cores = plan["n_cores"]
    N = plan["N"]
    nblk = plan["nblk"]
    npc = plan["npc"]
    npc_pad = plan["npc_pad"]
    cnt_blk = plan["cnt_blk"]
    chunk_start = plan["chunk_start"]
    total_chunks = plan["total_chunks"]
    groups = plan["groups"]
    rows_total = n_cores * npc_pad
    rg = [list(range(n_cores))]
    gcmax = max(gc for _, _, _, gc in groups)
    cbmax = int(cnt_blk.max())

    nc = bacc.Bacc("TRN2", target_bir_lowering=False, debug=False,
                   num_devices=n_cores, num_swdge_queues=4)

    # ---- I/O ----
    aT0 = nc.dram_tensor("aT0", [P, npc_pad], BF16, kind="ExternalInput")
    W1 = nc.dram_tensor("W1", [P, F1], BF16, kind="ExternalInput")
    W2 = nc.dram_tensor("W2", [P, F2], BF16, kind="ExternalInput")
    W3 = nc.dram_tensor("W3", [P, F3], BF16, kind="ExternalInput")
    g1 = nc.dram_tensor("g1", [P, 1], F32, kind="ExternalInput")
    be1 = nc.dram_tensor("be1", [P, 1], F32, kind="ExternalInput")
    g2 = nc.dram_tensor("g2", [P, 1], F32, kind="ExternalInput")
    be2 = nc.dram_tensor("be2", [P, 1], F32, kind="ExternalInput")
    b3b = nc.dram_tensor("b3b", [P, F3], F32, kind="ExternalInput")
    iota_in = nc.dram_tensor("iota", [P, P], I8, kind="ExternalInput")
    ident_in = nc.dram_tensor("ident", [P, P], BF16, kind="ExternalInput")
    idx_in = nc.dram_tensor("idx", [P, total_chunks * 8], I16,
                            kind="ExternalInput")
    dl_in = nc.dram_tensor("dl", [P, 2 * total_chunks], I8,
                           kind="ExternalInput")
    dinv_in = nc.dram_tensor("dinv_own", [P, nblk], F32, kind="ExternalInput")
    out_t = nc.dram_tensor("out", [npc_pad, F3], BF16, kind="ExternalOutput")

    with tile.TileContext(nc) as tc, ExitStack() as ctx:
        nc.gpsimd.load_library(library_config.mlp)

        sb = ctx.enter_context(tc.tile_pool(name="sb", bufs=1))
        # persistent sbuf state
        aT_a = sb.tile([P, npc_pad], BF16, tag="aT_a")
        aT_b = sb.tile([P, npc_pad], BF16, tag="aT_b")
        u_own = sb.tile([P, nblk, max(F1, F2)], F32, tag="u_own")
        u_own3 = sb.tile([P, nblk, F3], F32, tag="u_own3")
        z_own = sb.tile([P, nblk, max(F1, F2)], BF16, tag="z_own")
        w_sb = sb.tile([P, F1 + F2 + F3], BF16, tag="w_sb")
        iota8_t = sb.tile([P, P], I8, tag="iota8_t")
        ident_t = sb.tile([P, P], BF16, tag="ident_t")
        ones_t = sb.tile([P, 1], BF16, tag="ones_t")
        dinv_t = sb.tile([P, nblk], F32, tag="dinv_t")
        dl8_t = sb.tile([P, 2 * total_chunks], I8, tag="dl8_t")
        idx_t = sb.tile([P, total_chunks * 8], I16, tag="idx_t")
        bnp_t = sb.tile([P, 4], F32, tag="bnp_t")  # g1 be1 g2 be2
        b3_t = sb.tile([P, F3], F32, tag="b3_t")

        nc.sync.dma_start(w_sb[:, 0:F1], W1[:])
        nc.sync.dma_start(w_sb[:, F1:F1 + F2], W2[:])
        nc.sync.dma_start(w_sb[:, F1 + F2:], W3[:])
        nc.sync.dma_start(iota8_t[:], iota_in[:])
        nc.sync.dma_start(ident_t[:], ident_in[:])
        nc.sync.dma_start(dinv_t[:], dinv_in[:])
        nc.sync.dma_start(dl8_t[:], dl_in[:])
        nc.sync.dma_start(idx_t[:], idx_in[:])
        nc.sync.dma_start(bnp_t[:, 0:1], g1[:])
        nc.sync.dma_start(bnp_t[:, 1:2], be1[:])
        nc.sync.dma_start(bnp_t[:, 2:3], g2[:])
        nc.sync.dma_start(bnp_t[:, 3:4], be2[:])
        nc.sync.dma_start(b3_t[:], b3b[:])
        nc.gpsimd.memset(ones_t[:], 1.0)

        # DRAM scratch
        dram = ctx.enter_context(tc.tile_pool(name="dram", bufs=1,
                                              space="DRAM"))
        TDT = FP8 if table8 else BF16  # gather-table dtype, layers 1-2
        u1_dram = dram.tile([npc_pad, F1], TDT, tag="u1")
        u2_dram = dram.tile([npc_pad, F2], TDT, tag="u2")
        u3_dram = dram.tile([npc_pad, F3], BF16, tag="u3")
        st_in1 = dram.tile([P, 2], F32, tag="st_in1")
        st_in2 = dram.tile([P, 2], F32, tag="st_in2")

        # working pools
        psum_mm = ctx.enter_context(
            tc.tile_pool(name="psum_mm", bufs=2, space="PSUM"))
        psum_agg = ctx.enter_context(
            tc.tile_pool(name="psum_agg", bufs=2, space="PSUM"))
        psum_st = ctx.enter_context(
            tc.tile_pool(name="psum_st", bufs=1, space="PSUM"))
        spool = ctx.enter_context(tc.tile_pool(name="spool", bufs=4))
        gpool = ctx.enter_context(tc.tile_pool(name="gpool", bufs=4))
        g3pool = ctx.enter_context(tc.tile_pool(name="g3pool", bufs=2))
        tpool = ctx.enter_context(tc.tile_pool(name="tpool", bufs=3))
        qctr = [0]  # round-robin SWDGE queue for the gathers

        def layer(l, aT_in, aT_out, F_in, F_out, w_off, u_dram, ufull,
                  is_last, g_col=None, be_col=None, st_in=None, st_out=None):
            # ---------------- Phase A: dense matmul + u table ----------
            uo = u_own3 if is_last else u_own
            for b in range(nblk):
                h_ps = psum_mm.tile([P, F_out], F32, tag="mm")
                nc.tensor.matmul(
                    h_ps[:],
                    lhsT=aT_in[:, b * P:(b + 1) * P],
                    rhs=w_sb[:, w_off:w_off + F_out],
                    start=True, stop=True,
                )
                nc.scalar.activation(uo[:, b, :F_out], h_ps[:], AF.Copy,
                                     scale=dinv_t[:, b:b + 1])
            nc.gpsimd.dma_start(
                u_dram[:].rearrange("(b p) f -> p b f", p=P),
                uo[:, :, :F_out],
            )
            if n_cores > 1 and "nocoll" not in skip:
                nc.gpsimd.collective_compute(
                    "AllGather", ALU.bypass, replica_groups=rg,
                    ins=[u_dram[:].opt()], outs=[ufull[:].opt()],
                )
            else:
                nc.sync.dma_start(ufull[0:npc_pad, :], u_dram[:])

            # pair view of the gather table: one 2*F_out element = 2 rows
            upairs = ufull[:].rearrange("(r two) f -> r (two f)", two=2)

            # ---------------- Phase B: gather + segment matmul ---------
            if not is_last:
                st_s = psum_st.tile([P, 1], F32, tag="st_s")
                st_q = psum_st.tile([P, 1], F32, tag="st_q")
            tdt = BF16 if is_last else TDT
            for b0, g, cstart, g_chunks in groups:
                n_idx = g_chunks * P
                pool = g3pool if is_last else gpool
                gt = pool.tile([P, gcmax, 2 * F_out], tdt,
                               tag="g3" if is_last else "g")
                if "seqload" in skip:
                    nc.gpsimd.dma_start(
                        gt[:, :g_chunks, :2 * F_out],
                        upairs[0:n_idx, :].rearrange(
                            "(c p) f -> p c f", p=P),
                    )
                else:
                    nc.gpsimd.dma_gather(
                        gt[:, :g_chunks, :2 * F_out], upairs,
                        idx_t[:, cstart * 8:cstart * 8 + n_idx // 16],
                        n_idx, n_idx, 2 * F_out, single_packet=False,
                        queue_num=qctr[0] % 4,
                    )
                    qctr[0] += 1
                for bb in range(g):
                    b = b0 + bb
                    cb = int(cnt_blk[b])
                    c0 = int(chunk_start[b])
                    cl = c0 - cstart  # chunk offset within this gather
                    agg = psum_agg.tile([P, F_out], F32, tag="agg")
                    if "seg" in skip:
                        nc.vector.memset(agg[:], 0.0)
                    else:
                        s_w = spool.tile([P, 2 * cbmax, P], tdt,
                                         tag="s3" if is_last else "s")
                        nc.vector.tensor_tensor(
                            out=s_w[:, :2 * cb, :],
                            in0=iota8_t[:, None, :].to_broadcast(
                                [P, 2 * cb, P]),
                            in1=dl8_t[:, 2 * c0:2 * (c0 + cb)]
                            .to_broadcast([P, 2 * cb, P]),
                            op=ALU.is_equal,
                        )
                        if is_last:
                            for j in range(2 * cb):
                                ci, h = divmod(j, 2)
                                rhs = gt[:, cl + ci,
                                         h * F_out:(h + 1) * F_out]
                                nc.tensor.matmul(
                                    agg[:], lhsT=s_w[:, j, :], rhs=rhs,
                                    start=(j == 0), stop=(j == 2 * cb - 1),
                                )
                        else:
                            # fp8 DoubleRow: one instruction computes
                            # S_even^T @ g_even + S_odd^T @ g_odd
                            for ci in range(cb):
                                rhs = gt[:, cl + ci, :].rearrange(
                                    "p (two f) -> p two f", two=2)
                                nc.tensor.matmul(
                                    agg[:],
                                    lhsT=s_w[:, 2 * ci:2 * ci + 2, :],
                                    rhs=rhs,
                                    start=(ci == 0), stop=(ci == cb - 1),
                                    perf_mode=mybir.MatmulPerfMode.DoubleRow,
                                )
                    # epilogue: z = dinv * (agg + u_own)
                    uo = u_own3 if is_last else u_own
                    t_t = tpool.tile([P, F_out], F32, tag="t")
                    nc.vector.tensor_tensor(
                        out=t_t[:], in0=agg[:], in1=uo[:, b, :F_out],
                        op=ALU.add,
                    )
                    if is_last:
                        z3 = tpool.tile([P, F_out], F32, tag="z3")
                        nc.scalar.activation(z3[:], t_t[:], AF.Copy,
                                             scale=dinv_t[:, b:b + 1])
                        o_t = tpool.tile([P, F_out], BF16, tag="o")
                        nc.vector.tensor_tensor(out=o_t[:], in0=z3[:],
                                                in1=b3_t[:], op=ALU.add)
                        nc.sync.dma_start(out_t[b * P:(b + 1) * P, :],
                                          o_t[:])
                    else:
                        nc.scalar.activation(z_own[:, b, :F_out], t_t[:],
                                             AF.Copy,
                                             scale=dinv_t[:, b:b + 1])
                        z2 = tpool.tile([P, F_out], BF16, tag="z2")
                        nc.scalar.activation(z2[:], z_own[:, b, :F_out],
                                             AF.Square)
                        nc.tensor.matmul(st_s[:], lhsT=z_own[:, b, :F_out],
                                         rhs=ones_t[:],
                                         start=(b == 0), stop=(b == nblk - 1),
                                         skip_group_check=True)
                        nc.tensor.matmul(st_q[:], lhsT=z2[:], rhs=ones_t[:],
                                         start=(b == 0), stop=(b == nblk - 1),
                                         skip_group_check=True)
            if is_last:
                return

            # ---------------- Phase C: BN stats allreduce + coeffs -----
            st_sb = tpool.tile([P, 2], F32, tag="stsb")
            nc.vector.tensor_copy(st_sb[:, 0:1], st_s[:])
            nc.vector.tensor_copy(st_sb[:, 1:2], st_q[:])
            nc.sync.dma_start(st_in[:], st_sb[:])
            if n_cores > 1 and "nocoll" not in skip:
                nc.gpsimd.collective_compute(
                    "AllReduce", ALU.add, replica_groups=rg,
                    ins=[st_in[:].opt()], outs=[st_out[:].opt()],
                )
            else:
                nc.sync.dma_start(st_out[:], st_in[:])
            st_g = tpool.tile([P, 2], F32, tag="stg")
            nc.sync.dma_start(st_g[:], st_out[:])
            m_t = tpool.tile([P, 1], F32, tag="m")
            nc.scalar.activation(m_t[:], st_g[:, 0:1], AF.Copy, scale=1.0 / N)
            q_t = tpool.tile([P, 1], F32, tag="q")
            nc.scalar.activation(q_t[:], st_g[:, 1:2], AF.Copy, scale=1.0 / N)
            m2_t = tpool.tile([P, 1], F32, tag="m2")
            nc.scalar.activation(m2_t[:], m_t[:], AF.Square)
            v_t = tpool.tile([P, 1], F32, tag="v")
            nc.vector.tensor_tensor(out=v_t[:], in0=q_t[:], in1=m2_t[:],
                                    op=ALU.subtract)
            ve_t = tpool.tile([P, 1], F32, tag="ve")
            nc.vector.tensor_scalar(out=ve_t[:], in0=v_t[:], scalar1=1e-5,
                                    scalar2=None, op0=ALU.add)
            sd_t = tpool.tile([P, 1], F32, tag="sd")
            nc.scalar.activation(sd_t[:], ve_t[:], AF.Sqrt)
            inv_t = tpool.tile([P, 1], F32, tag="inv")
            nc.vector.reciprocal(inv_t[:], sd_t[:])
            a_t = tpool.tile([P, 1], F32, tag="A")
            nc.vector.tensor_tensor(out=a_t[:], in0=bnp_t[:, g_col:g_col + 1],
                                    in1=inv_t[:], op=ALU.mult)
            ma_t = tpool.tile([P, 1], F32, tag="mA")
            nc.vector.tensor_tensor(out=ma_t[:], in0=m_t[:], in1=a_t[:],
                                    op=ALU.mult)
            bb_t = tpool.tile([P, 1], F32, tag="B")
            nc.vector.tensor_tensor(out=bb_t[:],
                                    in0=bnp_t[:, be_col:be_col + 1],
                                    in1=ma_t[:], op=ALU.subtract)

            # ---------------- Phase D: transpose + BN apply + relu -----
            for b in range(nblk):
                zT = psum_mm.tile([P, P], BF16, tag="mmT")
                nc.tensor.transpose(zT[:], z_own[:, b, :F_out], ident_t[:])
                nc.scalar.activation(aT_out[:, b * P:(b + 1) * P], zT[:],
                                     AF.Relu, bias=bb_t[:], scale=a_t[:])

        for _rep in range(repeat):
            # collective outputs (Shared) allow only one writer instruction,
            # so each unrolled iteration gets its own set
            ufull1 = dram.tile([rows_total, F1], TDT, tag=f"uf1_{_rep}",
                               addr_space="Shared")
            ufull2 = dram.tile([rows_total, F2], TDT, tag=f"uf2_{_rep}",
                               addr_space="Shared")
            ufull3 = dram.tile([rows_total, F3], BF16, tag=f"uf3_{_rep}",
                               addr_space="Shared")
            st_out1 = dram.tile([P, 2], F32, tag=f"st_out1_{_rep}",
                                addr_space="Shared")
            st_out2 = dram.tile([P, 2], F32, tag=f"st_out2_{_rep}",
                                addr_space="Shared")
            nc.sync.dma_start(aT_a[:], aT0[:])
            layer(1, aT_a, aT_b, DIN, F1, 0, u1_dram, ufull1, False, 0, 1,
                  st_in1, st_out1)
            layer(2, aT_b, aT_a, F1, F2, F1, u2_dram, ufull2, False, 2, 3,
                  st_in2, st_out2)
            layer(3, aT_a, None, F2, F3, F1 + F2, u3_dram, ufull3, True)

    nc.compile()
    return nc


# ----------------------------------------------------------------------------
# Host entry point
# ----------------------------------------------------------------------------
def make_in_maps(plan, inputs, DIN, F1, F2, F3):
    import ml_dtypes

    n_cores = plan["n_cores"]
    npc = plan["npc"]
    npc_pad = plan["npc_pad"]
    x = np.asarray(inputs["x"], dtype=np.float32)
    bf = ml_dtypes.bfloat16
    iota = np.tile(np.arange(P)[None, :], (P, 1)).astype(np.int8)
    ident = np.eye(P, dtype=bf)
    b3b = np.tile(np.asarray(inputs["b3"], np.float32)[None, :], (P, 1))
    col = lambda v: np.asarray(v, np.float32).reshape(P, 1)
    W1 = np.asarray(inputs["W1"], np.float32).astype(bf)
    W2 = np.asarray(inputs["W2"], np.float32).astype(bf)
    W3 = np.asarray(inputs["W3"], np.float32).astype(bf)
    pos_of = plan["pos_of"]
    core_of = plan["core_of"]
    xbf = x.T.astype(bf)  # [DIN, N]
    in_maps = []
    for r in range(n_cores):
        sel = core_of == r
        aT0 = np.zeros((P, npc_pad), bf)
        aT0[:, pos_of[sel]] = xbf[:, sel]
        in_maps.append({
            "aT0": aT0,
            "W1": W1, "W2": W2, "W3": W3,
            "g1": col(inputs["g1"]), "be1": col(inputs["be1"]),
            "g2": col(inputs["g2"]), "be2": col(inputs["be2"]),
            "b3b": b3b, "iota": iota, "ident": ident,
            "idx": plan["idx"][r],
            "dl": plan["dl8"][r], "dinv_own": plan["dinv_own"][r],
        })
    return in_maps


# ----------------------------------------------------------------------------
# Cached PJRT execution (axon path): jit once, stage inputs once by content
# hash, keep the output-seed buffers on device permanently (the neuron
# lowering binds NEFF outputs to the custom-call result buffers; the seed
# operands are never read, they only exist to satisfy the calling convention).
# ----------------------------------------------------------------------------
class _Exec:
    def __init__(self, nc, n_cores):
        import jax
        import concourse.bass2jax as b2j
        from jax.sharding import Mesh, NamedSharding, PartitionSpec
        from jax.experimental.shard_map import shard_map

        b2j.install_neuronx_cc_hook()
        self.jax = jax
        self.nc = nc
        self.n_cores = n_cores
        pname = nc.partition_id_tensor.name if nc.partition_id_tensor else None
        in_names, out_names, out_avals = [], [], []
        for alloc in nc.m.functions[0].allocations:
            if not isinstance(alloc, mybir.MemoryLocationSet):
                continue
            name = alloc.memorylocations[0].name
            if alloc.kind == "ExternalInput":
                if name != pname:
                    in_names.append(name)
            elif alloc.kind == "ExternalOutput":
                out_names.append(name)
                out_avals.append(jax.core.ShapedArray(
                    tuple(alloc.tensor_shape), mybir.dt.np(alloc.dtype)))
        self.in_names = in_names
        self.out_names = out_names
        self.out_avals = out_avals
        n_params = len(in_names)
        bind_names = tuple(in_names + out_names + ([pname] if pname else []))

        def _body(*args):
            operands = list(args)
            if pname is not None:
                operands.append(b2j.partition_id_tensor())
            return tuple(b2j._bass_exec_p.bind(
                *operands,
                out_avals=tuple(out_avals),
                in_names=bind_names,
                out_names=tuple(out_names),
                lowering_input_output_aliases=(),
                sim_require_finite=True,
                sim_require_nnan=True,
                nc=nc,
            ))

        devices = jax.devices()[:n_cores]
        self.mesh = Mesh(np.asarray(devices), ("core",))
        self.sh = NamedSharding(self.mesh, PartitionSpec("core"))
        nin = n_params + len(out_names)
        self.fn = jax.jit(
            shard_map(
                _body, mesh=self.mesh,
                in_specs=(PartitionSpec("core"),) * nin,
                out_specs=(PartitionSpec("core"),) * len(out_names),
                check_rep=False,
            ),
            keep_unused=True,
        )
        self._staged = {}
        self._seeds = None

    def stage_inputs(self, in_maps):
        devs = []
        for name in self.in_names:
            arr = np.ascontiguousarray(
                np.concatenate([np.asarray(m[name]) for m in in_maps], axis=0))
            dig = hashlib.blake2b(arr.tobytes(), digest_size=16).digest()
            ent = self._staged.get(name)
            if ent is None or ent[0] != dig:
                ent = (dig, self.jax.device_put(arr, self.sh))
                self._staged[name] = ent
            devs.append(ent[1])
        return devs

    def seeds(self):
        if self._seeds is None:
            self._seeds = [
                self.jax.device_put(
                    np.zeros((self.n_cores * a.shape[0], *a.shape[1:]),
                             a.dtype), self.sh)
                for a in self.out_avals
            ]
            self.jax.block_until_ready(self._seeds)
        return self._seeds

    def run_staged(self, devs):
        return self.fn(*devs, *self.seeds())

    def run(self, in_maps):
        outs = self.run_staged(self.stage_inputs(in_maps))
        return [np.asarray(o) for o in outs]


_CACHE = {}


def _get_entry(inputs):
    x = np.asarray(inputs["x"], dtype=np.float32)
    N, DIN = x.shape
    F1 = inputs["W1"].shape[1]
    F2 = inputs["W2"].shape[1]
    F3 = inputs["W3"].shape[1]
    edge_index = np.asarray(inputs["edge_index"])
    key = (N, DIN, F1, F2, F3, hash(edge_index.tobytes()))
    if key not in _CACHE:
        plan = make_plan(edge_index, N)
        nc = build_kernel(plan, DIN, F1, F2, F3)
        ex = _Exec(nc, plan["n_cores"])
        _CACHE[key] = (plan, nc, ex)
    return _CACHE[key]


def kernel(**inputs):
    plan, nc, ex = _get_entry(inputs)
    F3 = inputs["W3"].shape[1]
    in_maps = make_in_maps(plan, inputs, inputs["x"].shape[1],
                           inputs["W1"].shape[1], inputs["W2"].shape[1], F3)
    outs = ex.run(in_maps)
    # "out" is concatenated over cores along axis 0: [n_cores*npc_pad, F3]
    # in permuted slot order; undo the load-balance permutation
    raw = outs[0].astype(np.float32).reshape(plan["n_cores"],
                                             plan["npc_pad"], F3)
    return raw[plan["core_of"], plan["pos_of"]]


if __name__ == "__main__":
    import reference

    inputs = {k: np.asarray(v) for k, v in reference.setup_inputs().items()}
    out = kernel(**inputs)
    exp = np.asarray(reference.reference(**inputs))
    err = np.abs(out - exp).max() / (np.abs(exp).max() + 1e-30)
    print("Relative error:", err)


# revision 61
# speedup vs baseline: 1.0130x; 1.0130x over previous
"""Trainium2 Bass kernel for a 3-layer GCN (DeepGRL) on 8 NeuronCores.

Strategy (per the dst-partitioned sharding plan):
  - Nodes are sharded contiguously across the 8 cores; edges are owned by the
    core that owns their destination node.
  - Per layer:  h = a @ W  (dense matmul on PE, per-core own nodes),
    u = dinv * h is written to a DRAM table and AllGather'ed so every core
    holds the full [N, F] table.
  - Aggregation out_i = dinv_i * (sum_{e: dst=i} u[src_e] + u_i) + b is done
    per 128-dst-node block: edge source rows are fetched with the SWDGE
    dma_gather instruction (128 rows per chunk land on 128 partitions), and a
    one-hot "segment matrix" S (built on the vector engine from the dst-local
    ids with an is_equal compare against an iota row) maps edges to dst rows
    via a PE matmul accumulating in PSUM.
  - BatchNorm batch statistics (sum / sum-of-squares per feature) are computed
    with ones-vector matmuls and AllReduce'd across cores; BN apply + ReLU is
    fused into one scalar-engine activation during the transpose back to the
    feature-major layout the next layer's matmul needs.

dma_gather indices are int16, so the gathered table is addressed in two
halves (rows < HALF and rows >= HALF); every dst block's edge list is split
into a "lo" and a "hi" sublist, each padded to a multiple of 128.

Execution path: the compiled module and the jax/PJRT executable are cached
per (shapes, edge-structure) key; input arrays are content-hashed and staged
on device once, so repeat calls only dispatch the NEFF and fetch the output.
"""

import hashlib
import math
from contextlib import ExitStack

import numpy as np

import concourse.bacc as bacc
import concourse.bass as bass
import concourse.mybir as mybir
import concourse.tile as tile
from concourse import library_config

P = 128
F32 = mybir.dt.float32
BF16 = mybir.dt.bfloat16
FP8 = mybir.dt.float8e4  # e4m3: required by the DoubleRow matmul perf mode
I16 = mybir.dt.int16
I8 = mybir.dt.int8
AF = mybir.ActivationFunctionType
ALU = mybir.AluOpType


# ----------------------------------------------------------------------------
# Host-side graph preprocessing
# ----------------------------------------------------------------------------
def make_plan(edge_index, N, n_cores=8, gb=2):
    """Partition edges by destination core, build per-core gather index /
    segment-id arrays (compile-time constants of the kernel).

    Gather layout: the u table is addressed in PAIRS of adjacent rows (one
    512-byte dma_gather element covers rows 2i and 2i+1), which halves the
    descriptor count of the random gathers (the dominant cost) and keeps the
    int16 index range (<= rows_total/2 = 25088).  Each edge therefore carries
    a parity bit; the one-hot segment matrix is built per (chunk, parity).
    Chunk counts are exact per dst block (max over the 8 cores, since the
    BIR is shared)."""
    src = np.asarray(edge_index[0], dtype=np.int64)
    dst = np.asarray(edge_index[1], dtype=np.int64)
    E = src.shape[0]

    indeg = np.bincount(dst, minlength=N).astype(np.float64)
    deg = indeg + 1.0  # self loop
    dinv = (1.0 / np.sqrt(deg)).astype(np.float32)

    npc = N // n_cores
    assert npc * n_cores == N
    nblk = math.ceil(npc / P)
    npc_pad = nblk * P

    # ---- load-balance: nodes are assigned to (core, block, slot) by an LPT
    # greedy on in-degree over ALL blocks of all cores (the host un-permutes
    # the output, so node->core ownership is free).  Every block ends up
    # with ~E/(n_cores*nblk) edges, so the shared per-block chunk counts
    # (maxed over cores) are tight and the per-core work is balanced.
    import heapq

    core_of = np.empty(N, np.int64)  # node -> owning core
    pos_of = np.empty(N, np.int64)   # node -> slot position in its core
    degs = indeg.astype(np.int64)
    order_d = np.argsort(-degs, kind="stable")
    heap = [(0, g, 0) for g in range(n_cores * nblk)]  # (load, gblock, used)
    heapq.heapify(heap)
    for i in order_d:
        while True:
            load, g, used = heapq.heappop(heap)
            if used < P:
                break
        core_of[i] = g // nblk
        pos_of[i] = (g % nblk) * P + used
        heapq.heappush(heap, (load + degs[i], g, used + 1))

    # source row in the padded, permuted global table layout.  Table rows
    # are ordered (slot, block) so the phase-A table store is 128 contiguous
    # partition-major descriptors instead of 6272 row-interleaved ones.
    trow = (pos_of % P) * nblk + pos_of // P
    src_row = core_of[src] * npc_pad + trow[src]
    pair = src_row // 2
    parity = src_row % 2

    dst_core = core_of[dst]
    dpos = pos_of[dst]
    gblk = dst_core * nblk + dpos // P  # global block id, 0..n_cores*nblk-1
    d_in_blk = dpos % P

    order = np.lexsort((pair, gblk))
    pair_s = pair[order]
    par_s = parity[order]
    dl_s = d_in_blk[order]

    counts = np.bincount(gblk, minlength=n_cores * nblk).reshape(
        n_cores, nblk)
    # chunks per dst block: shared across cores (same BIR), exact otherwise
    cnt_blk = np.maximum(1, -(-counts // P)).max(axis=0)  # [nblk]
    chunk_start = np.concatenate([[0], np.cumsum(cnt_blk)])  # [nblk+1]
    total_chunks = int(chunk_start[-1])

    # per-core slot arrays (slot = chunk position; sentinel dl=300, idx=0)
    idx_all = np.zeros((n_cores, total_chunks * P), np.int16)
    dle = np.full((n_cores, total_chunks * P), 300.0, np.float32)
    dlo = np.full((n_cores, total_chunks * P), 300.0, np.float32)
    edge_start = np.concatenate([[0], np.cumsum(counts.reshape(-1))])
    for r in range(n_cores):
        for b in range(nblk):
            g = r * nblk + b
            n_e = counts[r, b]
            s0 = edge_start[g]
            o0 = chunk_start[b] * P
            sl = slice(o0, o0 + n_e)
            idx_all[r, sl] = pair_s[s0:s0 + n_e]
            pe = par_s[s0:s0 + n_e]
            dd = dl_s[s0:s0 + n_e]
            dle[r, sl] = np.where(pe == 0, dd, 300.0)
            dlo[r, sl] = np.where(pe == 1, dd, 300.0)

    # gather groups of up to `gb` blocks: (b0, g, cstart, g_chunks)
    groups = []
    b0 = 0
    while b0 < nblk:
        g = min(gb, nblk - b0)
        cstart = int(chunk_start[b0])
        g_chunks = int(chunk_start[b0 + g] - chunk_start[b0])
        groups.append((b0, g, cstart, g_chunks))
        b0 += g

    # int16 wrap layout per gather: idx i -> [i % 16, i // 16], replicated
    # to 128 partitions (8 groups of 16); one wrapped strip per group
    idx_w = np.zeros((n_cores, P, total_chunks * 8), np.int16)
    for r in range(n_cores):
        for (_, _, cstart, g_chunks) in groups:
            ids = idx_all[r, cstart * P:(cstart + g_chunks) * P]
            w = ids.reshape(-1, 16).T  # [16, n/16]
            idx_w[r][:, cstart * 8:cstart * 8 + w.shape[1]] = np.tile(
                w, (8, 1))

    # dl array: per chunk two columns (even parity, odd parity)
    dl_arr = np.empty((n_cores, P, 2 * total_chunks), np.float32)
    dl_arr[:, :, 0::2] = dle.reshape(n_cores, total_chunks, P).transpose(
        0, 2, 1)
    dl_arr[:, :, 1::2] = dlo.reshape(n_cores, total_chunks, P).transpose(
        0, 2, 1)

    # int8 form for the DVE one-hot compare (sentinel -1)
    dl8 = np.where(dl_arr == 300.0, -1.0, dl_arr).astype(np.int8)

    dinv_own = np.zeros((n_cores, npc_pad), dtype=np.float32)
    dinv_own[core_of, pos_of] = dinv
    dinv_own = dinv_own.reshape(n_cores, nblk, P).transpose(0, 2, 1)

    return dict(
        n_cores=n_cores,
        N=N,
        E=E,
        npc=npc,
        nblk=nblk,
        npc_pad=npc_pad,
        cnt_blk=cnt_blk,
        chunk_start=chunk_start,
        total_chunks=total_chunks,
        groups=groups,
        idx=idx_w,
        dl=dl_arr,
        dinv_own=dinv_own,
        pos_of=pos_of,
        core_of=core_of,
        dl8=dl8,
    )


# ----------------------------------------------------------------------------
# Kernel builder (same BIR for all cores; per-core data via input tensors)
# ----------------------------------------------------------------------------
def build_kernel(plan, DIN, F1, F2, F3, skip=(), repeat=1, table8=True):
    n_cores = plan["n_cores"]
    N = plan["N"]
    nblk = plan["nblk"]
    npc = plan["npc"]
    npc_pad = plan["npc_pad"]
    cnt_blk = plan["cnt_blk"]
    chunk_start = plan["chunk_start"]
    total_chunks = plan["total_chunks"]
    groups = plan["groups"]
    rows_total = n_cores * npc_pad
    rg = [list(range(n_cores))]
    gcmax = max(gc for _, _, _, gc in groups)
    cbmax = int(cnt_blk.max())

    nc = bacc.Bacc("TRN2", target_bir_lowering=False, debug=False,
                   num_devices=n_cores, num_swdge_queues=4)

    # ---- I/O ----
    aT0 = nc.dram_tensor("aT0", [P, npc_pad], BF16, kind="ExternalInput")
    W1 = nc.dram_tensor("W1", [P, F1], BF16, kind="ExternalInput")
    W2 = nc.dram_tensor("W2", [P, F2], BF16, kind="ExternalInput")
    W3 = nc.dram_tensor("W3", [P, F3], BF16, kind="ExternalInput")
    g1 = nc.dram_tensor("g1", [P, 1], F32, kind="ExternalInput")
    be1 = nc.dram_tensor("be1", [P, 1], F32, kind="ExternalInput")
    g2 = nc.dram_tensor("g2", [P, 1], F32, kind="ExternalInput")
    be2 = nc.dram_tensor("be2", [P, 1], F32, kind="ExternalInput")
    b3b = nc.dram_tensor("b3b", [P, F3], F32, kind="ExternalInput")
    iota_in = nc.dram_tensor("iota", [P, P], I8, kind="ExternalInput")
    ident_in = nc.dram_tensor("ident", [P, P], BF16, kind="ExternalInput")
    idx_in = nc.dram_tensor("idx", [P, total_chunks * 8], I16,
                            kind="ExternalInput")
    dl_in = nc.dram_tensor("dl", [P, 2 * total_chunks], I8,
                           kind="ExternalInput")
    dinv_in = nc.dram_tensor("dinv_own", [P, nblk], F32, kind="ExternalInput")
    out_t = nc.dram_tensor("out", [npc_pad, F3], BF16, kind="ExternalOutput")

    with tile.TileContext(nc) as tc, ExitStack() as ctx:
        nc.gpsimd.load_library(library_config.mlp)

        sb = ctx.enter_context(tc.tile_pool(name="sb", bufs=1))
        # persistent sbuf state
        aT_a = sb.tile([P, npc_pad], BF16, tag="aT_a")
        aT_b = sb.tile([P, npc_pad], BF16, tag="aT_b")
        u_own = sb.tile([P, nblk, max(F1, F2)], F32, tag="u_own")
        u_own3 = sb.tile([P, nblk, F3], F32, tag="u_own3")
        z_own = sb.tile([P, nblk, max(F1, F2)], BF16, tag="z_own")
        w_sb = sb.tile([P, F1 + F2 + F3], BF16, tag="w_sb")
        iota8_t = sb.tile([P, P], I8, tag="iota8_t")
        ident_t = sb.tile([P, P], BF16, tag="ident_t")
        ones_t = sb.tile([P, 1], BF16, tag="ones_t")
        dinv_t = sb.tile([P, nblk], F32, tag="dinv_t")
        dl8_t = sb.tile([P, 2 * total_chunks], I8, tag="dl8_t")
        idx_t = sb.tile([P, total_chunks * 8], I16, tag="idx_t")
        bnp_t = sb.tile([P, 4], F32, tag="bnp_t")  # g1 be1 g2 be2
        b3_t = sb.tile([P, F3], F32, tag="b3_t")
        o_all = sb.tile([P, nblk, F3], BF16, tag="o_all")

        nc.sync.dma_start(w_sb[:, 0:F1], W1[:])
        nc.sync.dma_start(w_sb[:, F1:F1 + F2], W2[:])
        nc.sync.dma_start(w_sb[:, F1 + F2:], W3[:])
        nc.sync.dma_start(iota8_t[:], iota_in[:])
        nc.sync.dma_start(ident_t[:], ident_in[:])
        nc.sync.dma_start(dinv_t[:], dinv_in[:])
        nc.sync.dma_start(dl8_t[:], dl_in[:])
        nc.sync.dma_start(idx_t[:], idx_in[:])
        nc.sync.dma_start(bnp_t[:, 0:1], g1[:])
        nc.sync.dma_start(bnp_t[:, 1:2], be1[:])
        nc.sync.dma_start(bnp_t[:, 2:3], g2[:])
        nc.sync.dma_start(bnp_t[:, 3:4], be2[:])
        nc.sync.dma_start(b3_t[:], b3b[:])
        nc.gpsimd.memset(ones_t[:], 1.0)

        # DRAM scratch
        dram = ctx.enter_context(tc.tile_pool(name="dram", bufs=1,
                                              space="DRAM"))
        TDT = FP8 if table8 else BF16  # gather-table dtype, layers 1-2
        u1_dram = dram.tile([npc_pad, F1], TDT, tag="u1")
        u2_dram = dram.tile([npc_pad, F2], TDT, tag="u2")
        u3_dram = dram.tile([npc_pad, F3], BF16, tag="u3")
        st_in1 = dram.tile([P, 2], F32, tag="st_in1")
        st_in2 = dram.tile([P, 2], F32, tag="st_in2")

        # working pools
        psum_mm = ctx.enter_context(
            tc.tile_pool(name="psum_mm", bufs=2, space="PSUM"))
        psum_agg = ctx.enter_context(
            tc.tile_pool(name="psum_agg", bufs=2, space="PSUM"))
        psum_st = ctx.enter_context(
            tc.tile_pool(name="psum_st", bufs=1, space="PSUM"))
        spool = ctx.enter_context(tc.tile_pool(name="spool", bufs=4))
        gpool = ctx.enter_context(tc.tile_pool(name="gpool", bufs=4))
        g3pool = ctx.enter_context(tc.tile_pool(name="g3pool", bufs=2))
        tpool = ctx.enter_context(tc.tile_pool(name="tpool", bufs=3))
        qctr = [0]  # round-robin SWDGE queue for the gathers

        def layer(l, aT_in, aT_out, F_in, F_out, w_off, u_dram, ufull,
                  is_last, g_col=None, be_col=None, st_in=None, st_out=None):
            # ---------------- Phase A: dense matmul + u table ----------
            uo = u_own3 if is_last else u_own
            for b in range(nblk):
                h_ps = psum_mm.tile([P, F_out], F32, tag="mm")
                nc.tensor.matmul(
                    h_ps[:],
                    lhsT=aT_in[:, b * P:(b + 1) * P],
                    rhs=w_sb[:, w_off:w_off + F_out],
                    start=True, stop=True,
                )
                nc.scalar.activation(uo[:, b, :F_out], h_ps[:], AF.Copy,
                                     scale=dinv_t[:, b:b + 1])
            nc.gpsimd.dma_start(
                u_dram[:].rearrange("(p b) f -> p b f", p=P),
                uo[:, :, :F_out],
            )
            if n_cores > 1 and "nocoll" not in skip:
                nc.gpsimd.collective_compute(
                    "AllGather", ALU.bypass, replica_groups=rg,
                    ins=[u_dram[:].opt()], outs=[ufull[:].opt()],
                )
            else:
                nc.sync.dma_start(ufull[0:npc_pad, :], u_dram[:])

            # pair view of the gather table: one 2*F_out element = 2 rows
            upairs = ufull[:].rearrange("(r two) f -> r (two f)", two=2)

            # ---------------- Phase B: gather + segment matmul ---------
            if not is_last:
                st_s = psum_st.tile([P, 1], F32, tag="st_s")
                st_q = psum_st.tile([P, 1], F32, tag="st_q")
            tdt = BF16 if is_last else TDT
            for b0, g, cstart, g_chunks in groups:
                n_idx = g_chunks * P
                pool = g3pool if is_last else gpool
                gt = pool.tile([P, gcmax, 2 * F_out], tdt,
                               tag="g3" if is_last else "g")
                if "seqload" in skip:
                    nc.gpsimd.dma_start(
                        gt[:, :g_chunks, :2 * F_out],
                        upairs[0:n_idx, :].rearrange(
                            "(c p) f -> p c f", p=P),
                    )
                else:
                    nc.gpsimd.dma_gather(
                        gt[:, :g_chunks, :2 * F_out], upairs,
                        idx_t[:, cstart * 8:cstart * 8 + n_idx // 16],
                        n_idx, n_idx, 2 * F_out, single_packet=False,
                        queue_num=qctr[0] % 4,
                    )
                    qctr[0] += 1
                for bb in range(g):
                    b = b0 + bb
                    cb = int(cnt_blk[b])
                    c0 = int(chunk_start[b])
                    cl = c0 - cstart  # chunk offset within this gather
                    agg = psum_agg.tile([P, F_out], F32, tag="agg")
                    if "seg" in skip:
                        nc.vector.memset(agg[:], 0.0)
                    else:
                        s_w = spool.tile([P, 2 * cbmax, P], tdt,
                                         tag="s3" if is_last else "s")
                        nc.vector.tensor_tensor(
                            out=s_w[:, :2 * cb, :],
                            in0=iota8_t[:, None, :].to_broadcast(
                                [P, 2 * cb, P]),
                            in1=dl8_t[:, 2 * c0:2 * (c0 + cb)]
                            .to_broadcast([P, 2 * cb, P]),
                            op=ALU.is_equal,
                        )
                        if is_last:
                            for j in range(2 * cb):
                                ci, h = divmod(j, 2)
                                rhs = gt[:, cl + ci,
                                         h * F_out:(h + 1) * F_out]
                                nc.tensor.matmul(
                                    agg[:], lhsT=s_w[:, j, :], rhs=rhs,
                                    start=(j == 0), stop=(j == 2 * cb - 1),
                                )
                        else:
                            # fp8 DoubleRow: one instruction computes
                            # S_even^T @ g_even + S_odd^T @ g_odd
                            for ci in range(cb):
                                rhs = gt[:, cl + ci, :].rearrange(
                                    "p (two f) -> p two f", two=2)
                                nc.tensor.matmul(
                                    agg[:],
                                    lhsT=s_w[:, 2 * ci:2 * ci + 2, :],
                                    rhs=rhs,
                                    start=(ci == 0), stop=(ci == cb - 1),
                                    perf_mode=mybir.MatmulPerfMode.DoubleRow,
                                )
                    # epilogue: z = dinv * (agg + u_own)
                    uo = u_own3 if is_last else u_own
                    t_t = tpool.tile([P, F_out], F32, tag="t")
                    nc.vector.tensor_tensor(
                        out=t_t[:], in0=agg[:], in1=uo[:, b, :F_out],
                        op=ALU.add,
                    )
                    if is_last:
                        z3 = tpool.tile([P, F_out], F32, tag="z3")
                        nc.scalar.activation(z3[:], t_t[:], AF.Copy,
                                             scale=dinv_t[:, b:b + 1])
                        nc.vector.tensor_tensor(out=o_all[:, b, :],
                                                in0=z3[:],
                                                in1=b3_t[:], op=ALU.add)
                    else:
                        nc.scalar.activation(z_own[:, b, :F_out], t_t[:],
                                             AF.Copy,
                                             scale=dinv_t[:, b:b + 1])
                        z2 = tpool.tile([P, F_out], BF16, tag="z2")
                        nc.scalar.activation(z2[:], z_own[:, b, :F_out],
                                             AF.Square)
                        nc.tensor.matmul(st_s[:], lhsT=z_own[:, b, :F_out],
                                         rhs=ones_t[:],
                                         start=(b == 0), stop=(b == nblk - 1),
                                         skip_group_check=True)
                        nc.tensor.matmul(st_q[:], lhsT=z2[:], rhs=ones_t[:],
                                         start=(b == 0), stop=(b == nblk - 1),
                                         skip_group_check=True)
            if is_last:
                return

            # ---------------- Phase C: BN stats allreduce + coeffs -----
            st_sb = tpool.tile([P, 2], F32, tag="stsb")
            nc.vector.tensor_copy(st_sb[:, 0:1], st_s[:])
            nc.vector.tensor_copy(st_sb[:, 1:2], st_q[:])
            nc.sync.dma_start(st_in[:], st_sb[:])
            if n_cores > 1 and "nocoll" not in skip:
                nc.gpsimd.collective_compute(
                    "AllReduce", ALU.add, replica_groups=rg,
                    ins=[st_in[:].opt()], outs=[st_out[:].opt()],
                )
            else:
                nc.sync.dma_start(st_out[:], st_in[:])
            st_g = tpool.tile([P, 2], F32, tag="stg")
            nc.sync.dma_start(st_g[:], st_out[:])
            m_t = tpool.tile([P, 1], F32, tag="m")
            nc.scalar.activation(m_t[:], st_g[:, 0:1], AF.Copy, scale=1.0 / N)
            q_t = tpool.tile([P, 1], F32, tag="q")
            nc.scalar.activation(q_t[:], st_g[:, 1:2], AF.Copy, scale=1.0 / N)
            m2_t = tpool.tile([P, 1], F32, tag="m2")
            nc.scalar.activation(m2_t[:], m_t[:], AF.Square)
            v_t = tpool.tile([P, 1], F32, tag="v")
            nc.vector.tensor_tensor(out=v_t[:], in0=q_t[:], in1=m2_t[:],
                                    op=ALU.subtract)
            ve_t = tpool.tile([P, 1], F32, tag="ve")
            nc.vector.tensor_scalar(out=ve_t[:], in0=v_t[:], scalar1=1e-5,
                                    scalar2=None, op0=ALU.add)
            sd_t = tpool.tile([P, 1], F32, tag="sd")
            nc.scalar.activation(sd_t[:], ve_t[:], AF.Sqrt)
            inv_t = tpool.tile([P, 1], F32, tag="inv")
            nc.vector.reciprocal(inv_t[:], sd_t[:])
            a_t = tpool.tile([P, 1], F32, tag="A")
            nc.vector.tensor_tensor(out=a_t[:], in0=bnp_t[:, g_col:g_col + 1],
                                    in1=inv_t[:], op=ALU.mult)
            ma_t = tpool.tile([P, 1], F32, tag="mA")
            nc.vector.tensor_tensor(out=ma_t[:], in0=m_t[:], in1=a_t[:],
                                    op=ALU.mult)
            bb_t = tpool.tile([P, 1], F32, tag="B")
            nc.vector.tensor_tensor(out=bb_t[:],
                                    in0=bnp_t[:, be_col:be_col + 1],
                                    in1=ma_t[:], op=ALU.subtract)

            # ---------------- Phase D: transpose + BN apply + relu -----
            for b in range(nblk):
                zT = psum_mm.tile([P, P], BF16, tag="mmT")
                nc.tensor.transpose(zT[:], z_own[:, b, :F_out], ident_t[:])
                nc.scalar.activation(aT_out[:, b * P:(b + 1) * P], zT[:],
                                     AF.Relu, bias=bb_t[:], scale=a_t[:])

        for _rep in range(repeat):
            # collective outputs (Shared) allow only one writer instruction,
            # so each unrolled iteration gets its own set
            ufull1 = dram.tile([rows_total, F1], TDT, tag=f"uf1_{_rep}",
                               addr_space="Shared")
            ufull2 = dram.tile([rows_total, F2], TDT, tag=f"uf2_{_rep}",
                               addr_space="Shared")
            ufull3 = dram.tile([rows_total, F3], BF16, tag=f"uf3_{_rep}",
                               addr_space="Shared")
            st_out1 = dram.tile([P, 2], F32, tag=f"st_out1_{_rep}",
                                addr_space="Shared")
            st_out2 = dram.tile([P, 2], F32, tag=f"st_out2_{_rep}",
                                addr_space="Shared")
            nc.sync.dma_start(aT_a[:], aT0[:])
            layer(1, aT_a, aT_b, DIN, F1, 0, u1_dram, ufull1, False, 0, 1,
                  st_in1, st_out1)
            layer(2, aT_b, aT_a, F1, F2, F1, u2_dram, ufull2, False, 2, 3,
                  st_in2, st_out2)
            layer(3, aT_a, None, F2, F3, F1 + F2, u3_dram, ufull3, True)
            nc.sync.dma_start(
                out_t[:].rearrange("(p b) f -> p b f", p=P), o_all[:])

    nc.compile()
    return nc


# ----------------------------------------------------------------------------
# Host entry point
# ----------------------------------------------------------------------------
def make_in_maps(plan, inputs, DIN, F1, F2, F3):
    import ml_dtypes

    n_cores = plan["n_cores"]
    npc = plan["npc"]
    npc_pad = plan["npc_pad"]
    x = np.asarray(inputs["x"], dtype=np.float32)
    bf = ml_dtypes.bfloat16
    iota = np.tile(np.arange(P)[None, :], (P, 1)).astype(np.int8)
    ident = np.eye(P, dtype=bf)
    b3b = np.tile(np.asarray(inputs["b3"], np.float32)[None, :], (P, 1))
    col = lambda v: np.asarray(v, np.float32).reshape(P, 1)
    W1 = np.asarray(inputs["W1"], np.float32).astype(bf)
    W2 = np.asarray(inputs["W2"], np.float32).astype(bf)
    W3 = np.asarray(inputs["W3"], np.float32).astype(bf)
    pos_of = plan["pos_of"]
    core_of = plan["core_of"]
    xbf = x.T.astype(bf)  # [DIN, N]
    in_maps = []
    for r in range(n_cores):
        sel = core_of == r
        aT0 = np.zeros((P, npc_pad), bf)
        aT0[:, pos_of[sel]] = xbf[:, sel]
        in_maps.append({
            "aT0": aT0,
            "W1": W1, "W2": W2, "W3": W3,
            "g1": col(inputs["g1"]), "be1": col(inputs["be1"]),
            "g2": col(inputs["g2"]), "be2": col(inputs["be2"]),
            "b3b": b3b, "iota": iota, "ident": ident,
            "idx": plan["idx"][r],
            "dl": plan["dl8"][r], "dinv_own": plan["dinv_own"][r],
        })
    return in_maps


# ----------------------------------------------------------------------------
# Cached PJRT execution (axon path): jit once, stage inputs once by content
# hash, keep the output-seed buffers on device permanently (the neuron
# lowering binds NEFF outputs to the custom-call result buffers; the seed
# operands are never read, they only exist to satisfy the calling convention).
# ----------------------------------------------------------------------------
class _Exec:
    def __init__(self, nc, n_cores):
        import jax
        import concourse.bass2jax as b2j
        from jax.sharding import Mesh, NamedSharding, PartitionSpec
        from jax.experimental.shard_map import shard_map

        b2j.install_neuronx_cc_hook()
        self.jax = jax
        self.nc = nc
        self.n_cores = n_cores
        pname = nc.partition_id_tensor.name if nc.partition_id_tensor else None
        in_names, out_names, out_avals = [], [], []
        for alloc in nc.m.functions[0].allocations:
            if not isinstance(alloc, mybir.MemoryLocationSet):
                continue
            name = alloc.memorylocations[0].name
            if alloc.kind == "ExternalInput":
                if name != pname:
                    in_names.append(name)
            elif alloc.kind == "ExternalOutput":
                out_names.append(name)
                out_avals.append(jax.core.ShapedArray(
                    tuple(alloc.tensor_shape), mybir.dt.np(alloc.dtype)))
        self.in_names = in_names
        self.out_names = out_names
        self.out_avals = out_avals
        n_params = len(in_names)
        bind_names = tuple(in_names + out_names + ([pname] if pname else []))

        def _body(*args):
            operands = list(args)
            if pname is not None:
                operands.append(b2j.partition_id_tensor())
            return tuple(b2j._bass_exec_p.bind(
                *operands,
                out_avals=tuple(out_avals),
                in_names=bind_names,
                out_names=tuple(out_names),
                lowering_input_output_aliases=(),
                sim_require_finite=True,
                sim_require_nnan=True,
                nc=nc,
            ))

        devices = jax.devices()[:n_cores]
        self.mesh = Mesh(np.asarray(devices), ("core",))
        self.sh = NamedSharding(self.mesh, PartitionSpec("core"))
        nin = n_params + len(out_names)
        self.fn = jax.jit(
            shard_map(
                _body, mesh=self.mesh,
                in_specs=(PartitionSpec("core"),) * nin,
                out_specs=(PartitionSpec("core"),) * len(out_names),
                check_rep=False,
            ),
            keep_unused=True,
        )
        self._staged = {}
        self._seeds = None

    def stage_inputs(self, in_maps):
        devs = []
        for name in self.in_names:
            arr = np.ascontiguousarray(
                np.concatenate([np.asarray(m[name]) for m in in_maps], axis=0))
            dig = hashlib.blake2b(arr.tobytes(), digest_size=16).digest()
            ent = self._staged.get(name)
            if ent is None or ent[0] != dig:
                ent = (dig, self.jax.device_put(arr, self.sh))
                self._staged[name] = ent
            devs.append(ent[1])
        return devs

    def seeds(self):
        if self._seeds is None:
            self._seeds = [
                self.jax.device_put(
                    np.zeros((self.n_cores * a.shape[0], *a.shape[1:]),
                             a.dtype), self.sh)
                for a in self.out_avals
            ]
            self.jax.block_until_ready(self._seeds)
        return self._seeds

    def run_staged(self, devs):
        return self.fn(*devs, *self.seeds())

    def run(self, in_maps):
        outs = self.run_staged(self.stage_inputs(in_maps))
        return [np.asarray(o) for o in outs]


_CACHE = {}


def _get_entry(inputs):
    x = np.asarray(inputs["x"], dtype=np.float32)
    N, DIN = x.shape
    F1 = inputs["W1"].shape[1]
    F2 = inputs["W2"].shape[1]
    F3 = inputs["W3"].shape[1]
    edge_index = np.asarray(inputs["edge_index"])
    key = (N, DIN, F1, F2, F3, hash(edge_index.tobytes()))
    if key not in _CACHE:
        plan = make_plan(edge_index, N)
        nc = build_kernel(plan, DIN, F1, F2, F3)
        ex = _Exec(nc, plan["n_cores"])
        _CACHE[key] = (plan, nc, ex)
    return _CACHE[key]


def kernel(**inputs):
    plan, nc, ex = _get_entry(inputs)
    F3 = inputs["W3"].shape[1]
    in_maps = make_in_maps(plan, inputs, inputs["x"].shape[1],
                           inputs["W1"].shape[1], inputs["W2"].shape[1], F3)
    outs = ex.run(in_maps)
    # "out" is concatenated over cores along axis 0: [n_cores*npc_pad, F3]
    # in permuted slot order; undo the load-balance permutation
    raw = outs[0].astype(np.float32).reshape(plan["n_cores"], P,
                                             plan["nblk"], F3)
    return raw[plan["core_of"], plan["pos_of"] % P, plan["pos_of"] // P]


if __name__ == "__main__":
    import reference

    inputs = {k: np.asarray(v) for k, v in reference.setup_inputs().items()}
    out = kernel(**inputs)
    exp = np.asarray(reference.reference(**inputs))
    err = np.abs(out - exp).max() / (np.abs(exp).max() + 1e-30)
    print("Relative error:", err)
